# revision 34
# baseline (speedup 1.0000x reference)
"""Trainium2 Bass kernel for the AdSBHNet holographic-potential problem.

Computes, for a batch of turning points zs (B=8192) and small polynomial
coefficient vectors a, b plus scalars logcoef/shift:

    L  = 4 zs/pi * trapz-integral over y of  sqrt(fs) * W2 * y * sqrt(gn/(gd*t1))
    Vc = 4 pi/zs * trapz of (y/W2) * S * (fz - sqrt(t1*fz))
    Vd = 2 pi (1-zs) * trapz of sqrt(fzd*gnd/(gdd*zd^4))
    out = [L, exp(logcoef)*(Vc - Vd) + shift]

Sharding: data-parallel over zs across 8 NeuronCores (1024 each). On each
core the 1000 quadrature points sit on SBUF partitions (8 tiles x 125) and
the 1024 zs values on the free dimension. All bilinear "polynomial" grids
(fz, gn, gd, t1, gdd, fzd, gnd) are low-rank in (y-coeffs x zs-power-rows)
and are built by TensorEngine matmuls straight into PSUM; DVE/ACT/GPSIMD do
the sqrt/div chain; the quadrature reductions are K=125 matmuls with the
trapezoid weights as the stationary operand, accumulated across y-tiles in
PSUM. Everything f32 (f32r on the PE).
"""

import math
import numpy as np

B_TOTAL = 8192
NCORES = 8
BPC = B_TOTAL // NCORES          # 1024 zs per core
S = 1000                         # quadrature steps
NT = 8                           # y tiles per core
P = S // NT                      # 125 partitions per y tile
HALF = 512                       # matmul moving free dim

_COMPILED = {}


def _trapz_weights(x, append_one):
    """Node weights on the raw integrand I_0..I_{S-1} reproducing the
    reference's trapz over [0, x..., (1)] with linear extrapolation to 0
    (and a zero appended at 1 when append_one)."""
    n = len(x)
    u = np.zeros(n)
    u[0] = 0.5 * (x[1] - 0.0)
    u[1:-1] = 0.5 * (x[2:] - x[:-2])
    if append_one:
        u[-1] = 0.5 * (1.0 - x[-2])
    else:
        u[-1] = 0.5 * (x[-1] - x[-2])
    w_i0 = 0.5 * x[0]
    d = x[1] - x[0]
    u[0] += w_i0 * (1.0 + x[0] / d)
    u[1] += w_i0 * (-x[0] / d)
    return u


def _build_host_tables(a, b, logcoef, shift, zs):
    """All small derived constants, in float64, cast to f32 at the end."""
    a = np.asarray(a, np.float64)
    b = np.asarray(b, np.float64)
    lc = float(np.asarray(logcoef).reshape(-1)[0])
    sh = float(np.asarray(shift).reshape(-1)[0])
    zs = np.asarray(zs, np.float64)

    y = np.linspace(0.001, 0.999, S)
    y2 = np.linspace(0.001, 1.0, S)

    fa1 = 4.0 / 3.0 * a[0]
    fa2 = 2.0 * a[1]
    fa4 = -(1.0 + fa1 + fa2)

    w = 1.0 - y * y
    W2 = w * w
    W4 = W2 * W2
    ones = np.ones(S)

    # lhsT coefficient blocks [8, 5, S] (block, K-row, y)
    lcoef = np.zeros((8, 5, S))
    # gn = 1 + b0*w*zs + b1*W2*zs^2          (rhs rows: 1, zs, zs2, zs4, fs)
    lcoef[0] = [ones, b[0] * w, b[1] * W2, 0 * w, 0 * w]
    # gd = 1 - W4*zs^4
    lcoef[1] = [ones, 0 * w, 0 * w, -W4, 0 * w]
    # fz = 1 + fa1*w*zs + fa2*W2*zs^2 + fa4*W4*zs^4
    lcoef[2] = [ones, fa1 * w, fa2 * W2, fa4 * W4, 0 * w]
    # t1 = fz - fs*W4, cancellation-free form (t1 -> 0 as y -> 0):
    #    = fa1*(w-1)*zs + fa2*(W2-1)*zs^2 + fa4*(W4-1)*zs^4 + (1-W4)*fs
    lcoef[3] = [0 * w, fa1 * (w - 1), fa2 * (W2 - 1), fa4 * (W4 - 1), 1.0 - W4]
    # Vd grids: zd = 1 + e*u, e = y2, u = zs-1   (rhs rows: 1, u, u2, u3, u4)
    e = y2
    # zd = 1 + e*u directly (the 1 - zd^4 binomial form suffers a
    # catastrophic cancellation at small zd under the PE's f32r rounding)
    lcoef[4] = [ones, e, 0 * e, 0 * e, 0 * e]
    # fzd = f(zd) = sum_j gam_j e^j u^j ; gam_0 = f(1) = 0 analytically
    g1 = fa1 + 2 * fa2 + 4 * fa4
    g2 = fa2 + 6 * fa4
    g3 = 4 * fa4
    g4 = fa4
    lcoef[5] = [0 * e, g1 * e, g2 * e**2, g3 * e**3, g4 * e**4]
    # gnd = 1 + b0*zd + b1*zd^2 = d0 + d1 e u + d2 e^2 u^2
    d0 = 1.0 + b[0] + b[1]
    d1 = b[0] + 2 * b[1]
    d2 = b[1]
    lcoef[6] = [d0 * ones, d1 * e, d2 * e**2, 0 * e, 0 * e]
    # mu = W4 * fs (exact, multiplicative; used for the cancellation-free
    # Vc term  fz - sqrt(t1*fz) = mu / (1 + sqrt(t1/fz)))
    lcoef[7] = [0 * w, 0 * w, 0 * w, 0 * w, W4]
    # -> [5, 7*NT*P]: K-rows on partitions (base 0); block-major, then
    # y-tile, then within-tile index along the free dim, so each matmul's
    # lhsT is lcoef[:, (b*NT+t)*P : (b*NT+t+1)*P]
    lcoef_t = (
        lcoef.reshape(8, 5, NT, P).transpose(1, 0, 2, 3).reshape(5, 8 * NT * P)
        .astype(np.float32).copy()
    )

    uL = _trapz_weights(y, append_one=True)
    uD = _trapz_weights(y2, append_one=False)
    # [P, NT*3 + 4]: tile t's three weight columns at cols 3t..3t+2, then
    # two 2-column partition-selector blocks for the tail (L/Vc gather and
    # a zero/Vd gather)
    rw = np.stack([uL * y * W2, uL * y / W2, uD], axis=1)  # [S, 3]
    rwts = np.zeros((P, NT * 3 + 4), np.float32)
    rwts[:, 0:NT * 3] = (
        rw.reshape(NT, P, 3).transpose(1, 0, 2).reshape(P, NT * 3)
    )
    rwts[0, NT * 3 + 0] = 1.0     # Sel1 col0 <- acc row 0  (L)
    rwts[32, NT * 3 + 1] = 1.0    # Sel1 col1 <- acc row 32 (Vc)
    rwts[64, NT * 3 + 3] = 1.0    # Sel2 col1 <- acc row 64 (Vd); col0 = 0

    # per-core zs-derived rows
    zrows_all = []
    srows_all = []
    for c in range(NCORES):
        z = zs[c * BPC:(c + 1) * BPC]
        z2 = z * z
        z4 = z2 * z2
        fs = 1.0 + fa1 * z + fa2 * z2 + fa4 * z4
        u = z - 1.0
        zrows = np.stack(
            [np.ones(BPC), z, z2, z4, fs, np.ones(BPC), u, u * u, u**3, u**4]
        ).astype(np.float32)
        scaleL = 4.0 / math.pi * z * np.sqrt(fs)
        sA = math.exp(lc) * 4.0 * math.pi / z
        sB = -math.exp(lc) * 2.0 * math.pi * (1.0 - z)
        srows = np.zeros((2, 3 * BPC))
        srows[0, 0:BPC] = scaleL
        srows[1, 0:BPC] = sA
        srows[1, BPC:2 * BPC] = sB
        srows[1, 2 * BPC:3 * BPC] = sh
        zrows_all.append(zrows)
        srows_all.append(srows.astype(np.float32))
    return lcoef_t, rwts, zrows_all, srows_all



def _patch_tile_drain():
    """Walrus rejects instructions with >4 sync waits; Tile's kernel-tail
    drain waits on every active processor at once. Split it into one drain
    per processor (SP-engine drains are ~12 ns each)."""
    import re as _re
    import concourse.tile as tile_mod
    import bass_rust
    from bass_rust import ScopedClock

    if getattr(tile_mod.TileContext, "_drain_patched", False):
        return

    def _patched(self, tick_clock, wait_clock):
        gc = tick_clock.global_clock
        ticks = [int(x) for x in _re.findall(r"\d+", repr(gc))]
        for i in [i for i, t in enumerate(ticks) if t > 0]:
            sub = bass_rust.VectorClock()
            sub.require_at_least(i, ticks[i])
            d = self.nc.sync.drain()
            wait_clock.add_sem_waits(d.ins, ScopedClock({None: sub}))
        self.nc.all_engine_barrier()
        popped = self.nc._tile_sem_poison_stack.pop()
        assert popped is self._sem_poison
        self.nc.clear_and_free_semaphores(list(self.sems.allocated().values()))
        self.nc.all_engine_barrier()

    tile_mod.TileContext._drain_and_barrier = _patched
    tile_mod.TileContext._drain_patched = True


def _prune_redundant_waits(nc):
    """Tile emits per-instruction sem waits that are not transitively minimal
    (syncing on engine X does not teach it what X itself had waited on), but
    every TPB instruction has exactly ONE sync-wait slot. Run a vector-clock
    closure over the scheduled program, drop every wait already implied by
    the instruction's processor, and hoist any excess waits onto earlier
    same-processor instructions with a free slot (cycle-checked)."""
    insts = []
    for blk in nc.m.functions[0].blocks:
        insts.extend(blk.instructions)

    # semaphores that ever decrease (barrier gather/release) are not
    # monotonic; never prune or reason transitively through them
    nonmono = set()
    for inst in insts:
        si = inst.sync_info
        if si is None:
            continue
        for u in si.on_update or []:
            if getattr(u, "sync_type", "") == "semaphore" and \
                    getattr(u, "update_mode", "") != "sem-inc":
                nonmono.add(u.id)
        for w in si.on_wait or []:
            nm = getattr(w, "ant_name", "") or ""
            if "barrier" in nm:
                nonmono.add(w.id)

    V = {}          # processor key -> {sem_id: observed value}
    snap = {}       # sem_id -> {value: dict snapshot}
    cnt = {}        # sem_id -> current value
    own_sem = {}    # processor key -> its own sem id
    # per processor: list of (sync_info, own_tick_at_emit) with a free slot
    free_slots = {}

    def proc_key(inst):
        si = inst.sync_info
        if si is not None:
            for u in si.on_update or []:
                nm = getattr(u, "ant_name", "") or ""
                if nm.startswith("DMA"):
                    return nm
        return str(inst.engine)

    def dep_state(sem, val):
        snaps = snap.get(sem)
        if not snaps:
            return None
        keys = [k for k in snaps if k >= val]
        if not keys:
            return None
        return snaps[min(keys)]

    def merge_from(state, sem, val):
        state[sem] = max(state.get(sem, 0), val)
        ds = dep_state(sem, val)
        if ds:
            for s2, v2 in ds.items():
                if state.get(s2, 0) < v2:
                    state[s2] = v2

    n_dropped = n_hoisted = n_left = 0
    for inst in insts:
        si = inst.sync_info
        pk = proc_key(inst)
        state = V.setdefault(pk, {})
        my_sem = own_sem.get(pk)
        if si is not None and si.on_wait:
            kept = []
            movable = []
            for w in si.on_wait:
                if getattr(w, "sync_type", "") != "semaphore" or \
                        getattr(w, "wait_mode", "") != "sem-ge-imm" or \
                        w.id in nonmono:
                    kept.append(w)
                    continue
                sem, val = w.id, w.wait_value
                if state.get(sem, 0) >= val:
                    n_dropped += 1
                else:
                    movable.append(w)
                merge_from(state, sem, val)
            # hoist all but one movable wait onto earlier free slots
            while len(kept) + len(movable) > 1 and movable:
                w = movable.pop(0)
                placed = False
                for tsi, ttick in reversed(free_slots.get(pk, [])):
                    ds = dep_state(w.id, w.wait_value) or {}
                    # the target's own completion is tick `ttick`; the
                    # producer may only depend on strictly earlier ticks
                    if my_sem is not None and ds.get(my_sem, 0) >= ttick:
                        continue  # would deadlock
                    if not ds:
                        continue  # unknown producer: don't risk it
                    tsi.on_wait = [w]
                    free_slots[pk].remove((tsi, ttick))
                    placed = True
                    n_hoisted += 1
                    break
                if not placed:
                    kept.append(w)
                    n_left += 1
            kept.extend(movable)
            if len(kept) != len(si.on_wait):
                si.on_wait = kept
        if si is not None:
            for u in si.on_update or []:
                if getattr(u, "sync_type", "") != "semaphore":
                    continue
                sem = u.id
                if getattr(u, "update_mode", "") != "sem-inc" or sem in nonmono:
                    continue
                uv = getattr(u, "update_value", 1) or 1
                cnt[sem] = cnt.get(sem, 0) + uv
                if not pk.startswith("DMA"):
                    own_sem.setdefault(pk, sem)
                here = dict(state)
                here[sem] = cnt[sem]
                snap.setdefault(sem, {})[cnt[sem]] = here
                state[sem] = cnt[sem]
        if (si is not None and not si.on_wait and not pk.startswith("DMA")
                and str(getattr(inst, "opcode", "")) not in ("Matmult",)):
            free_slots.setdefault(pk, []).append(
                (si, cnt.get(own_sem.get(pk, -1), 0)))
    if n_left:
        import logging
        logging.warning("_prune_redundant_waits: %d waits could not be "
                        "hoisted; compile may fail", n_left)
    return n_dropped, n_hoisted, n_left


def _act_raw(nc, mybir, func, out, in_, scale=1.0, bias=0.0):
    eng = nc.scalar
    return eng.add_instruction(mybir.InstActivation(
        name=nc.get_next_instruction_name(), func=func,
        ins=[eng.lower_ap(in_),
             mybir.ImmediateValue(dtype=mybir.dt.float32, value=bias),
             mybir.ImmediateValue(dtype=mybir.dt.float32, value=scale),
             mybir.ImmediateValue(dtype=mybir.dt.float32, value=0.0)],
        outs=[eng.lower_ap(out)]))


def _build_nc():
    import concourse.bass as bass
    import concourse.mybir as mybir
    from concourse.tile import TileContext
    from concourse.bass import _add_dep_helper

    f32 = mybir.dt.float32
    f32r = mybir.dt.float32r
    bf16 = mybir.dt.bfloat16
    AF = mybir.ActivationFunctionType

    _patch_tile_drain()
    nc = bass.Bass()
    zrows_d = nc.declare_dram_parameter("zrows", [10, BPC], f32, isOutput=False)
    lcoef_d = nc.declare_dram_parameter("lcoef", [5, 8 * NT * P], f32, isOutput=False)
    rwts_d = nc.declare_dram_parameter("rwts", [P, NT * 3 + 4], f32, isOutput=False)
    srows_d = nc.declare_dram_parameter("srows", [2, 3 * BPC], f32, isOutput=False)
    out_d = nc.declare_dram_parameter("out", [2, BPC], f32, isOutput=True)

    # The TPB ISA gives a Matmult exactly ONE sync-wait slot and other
    # engine instructions two, so dependencies are funneled: every tensor a
    # poly matmul touches is released by ACT, every reduce-matmul input is
    # DVE-produced, all constants are DMA'd once up front and their DMA
    # queues "absorbed" into PE/DVE clocks by dummy ops. GPSIMD is unused.
    with TileContext(nc) as tc:
        with (
            tc.tile_pool(name="const", bufs=1) as cp,
            tc.tile_pool(name="io", bufs=2) as iop,
            tc.tile_pool(name="work", bufs=2) as wp,
            tc.tile_pool(name="ps", bufs=3, space="PSUM") as pp,
            tc.tile_pool(name="acc", bufs=1, space="PSUM") as accp,
        ):
            # ---- preamble: all constants in one shot ----
            zrc = cp.tile([5, BPC], f32r)
            nc.sync.dma_start(out=zrc[:], in_=zrows_d[0:5, :].bitcast(f32r))
            zrd = cp.tile([5, BPC], f32r)
            nc.sync.dma_start(out=zrd[:], in_=zrows_d[5:10, :].bitcast(f32r))
            lco = cp.tile([5, 8 * NT * P], f32r)
            halfc = 8 * NT * P // 2
            nc.sync.dma_start(out=lco[:, 0:halfc],
                              in_=lcoef_d[:, 0:halfc].bitcast(f32r))
            nc.sync.dma_start(out=lco[:, halfc:],
                              in_=lcoef_d[:, halfc:].bitcast(f32r))
            rw = cp.tile([P, NT * 3 + 4], f32)
            nc.sync.dma_start(out=rw[:], in_=rwts_d[:])
            sr = cp.tile([2, 3 * BPC], f32)
            nc.sync.dma_start(out=sr[:], in_=srows_d[:])

            # accumulators: matmul outs must sit at partition base 0/32/64
            # (row 0 = L, row 32 = Vc, row 64 = Vd); dummy absorber matmuls
            # also write [0:1, 0:1] and are overwritten by the first real
            # start=True accumulation.
            acc = accp.tile([65, 2 * HALF], f32)

            # absorb each const DMA's HW queue into the PE clock so later
            # matmuls never need a DMA wait
            for nm, ap_ in (("zrc", zrc[:, 0:1]), ("zrd", zrd[:, 0:1]),
                            ("lcoA", lco[:, 0:1]), ("lcoB", lco[:, halfc:halfc + 1])):
                a32 = ap_.bitcast(f32)
                nc.tensor.matmul(acc[0:1, 0:1], a32, a32, start=True, stop=True,
                                 skip_group_check=True)
            nc.tensor.matmul(acc[0:1, 0:1], rw[:, 0:1], rw[:, 0:1],
                             start=True, stop=True, skip_group_check=True)
            # rows other than 0/32/64 are never written by the matmuls but
            # are read by the tail gather; zero the tile (after the absorber
            # dummies so their single wait slot stays free for the DMA sems)
            nc.vector.memset(acc[:], 0.0)
            # absorb the srows DMA queue into the DVE clock for the tail ops
            tinyv = cp.tile([1, 1], f32)
            nc.vector.tensor_copy(out=tinyv[:], in_=sr[0:1, 0:1])
            tinya = cp.tile([1, 1], f32)
            tinyp = cp.tile([1, 1], f32)

            def blk(b, t):
                return lco[:, (b * NT + t) * P:(b * NT + t + 1) * P]


            prev_st = None
            prev_QG = None
            prev_QGT = None
            for t in range(NT):
                first = t == 0
                last = t == NT - 1
                if prev_st is not None:
                    nc.scalar.copy(out=tinya[:], in_=prev_st[0:1, 0:1])
                if prev_QG is not None:
                    nc.scalar.copy(out=tinya[:], in_=prev_QG[0:1, 0:1])
                if prev_QGT is not None:
                    qg_abs = nc.gpsimd.tensor_copy(out=tinyp[:], in_=prev_QGT[0:1, 0:1])
                else:
                    qg_abs = None

                # ---- connected grids (y): gn, gd, fz, t1 via PE ----
                gn_ps = pp.tile([P, BPC], f32, tag="poly", name=f"gn_ps{t}")
                gd_ps = pp.tile([P, BPC], f32, tag="poly", name=f"gd_ps{t}")
                fz_ps = pp.tile([P, BPC], f32, tag="poly", name=f"fz_ps{t}")
                t1_ps = pp.tile([P, BPC], f32, tag="poly", name=f"t1_ps{t}")
                mu_ps = pp.tile([P, BPC], f32, tag="poly", name=f"mu_ps{t}")
                for h in range(2):
                    cs = slice(h * HALF, (h + 1) * HALF)
                    rz = zrc[:, cs]
                    nc.tensor.matmul(gn_ps[:, cs], blk(0, t), rz, start=True, stop=True)
                    nc.tensor.matmul(gd_ps[:, cs], blk(1, t), rz, start=True, stop=True)
                    nc.tensor.matmul(fz_ps[:, cs], blk(2, t), rz, start=True, stop=True)
                    nc.tensor.matmul(t1_ps[:, cs], blk(3, t), rz, start=True, stop=True)
                    nc.tensor.matmul(mu_ps[:, cs], blk(7, t), rz, start=True, stop=True)

                mu_sb = wp.tile([P, BPC], f32, tag="mu_sb", name=f"mu_sb{t}")
                nc.vector.tensor_copy(out=mu_sb[:], in_=mu_ps[:])
                gn_sb = wp.tile([P, BPC], f32, tag="gn_sb", name=f"gn_sb{t}")
                nc.scalar.copy(out=gn_sb[:], in_=gn_ps[:])
                gd_sb = wp.tile([P, BPC], f32, tag="gd_sb", name=f"gd_sb{t}")
                nc.scalar.copy(out=gd_sb[:], in_=gd_ps[:])
                fz_sb = wp.tile([P, BPC], f32, tag="fz_sb", name=f"fz_sb{t}")
                nc.scalar.copy(out=fz_sb[:], in_=fz_ps[:])

                QG = wp.tile([P, BPC], f32, tag="QG", name=f"QG{t}", bufs=1)
                qg_i = nc.gpsimd.tensor_mul(QG[:], gn_sb[:], gd_sb[:])
                if qg_abs is not None:
                    _add_dep_helper(qg_i.ins, qg_abs.ins, sync=False,
                                    reason="pool absorber order")
                prev_QG = QG
                t1_abs = nc.vector.tensor_copy(out=tinyv[:], in_=t1_ps[0:1, 0:1])
                QGT = wp.tile([P, BPC], f32, tag="QGT", name=f"QGT{t}")
                qgt_i = nc.vector.tensor_mul(QGT[:], QG[:], t1_ps[:])
                _add_dep_helper(qgt_i.ins, t1_abs.ins, sync=False,
                                reason="dve absorber order")
                prev_QGT = QGT
                rQ = wp.tile([P, BPC], f32, tag="rQ", name=f"rQ{t}")
                _act_raw(nc, mybir, AF.Rsqrt, rQ[:], QGT[:])
                Sg = wp.tile([P, BPC], bf16, tag="Sg", name=f"Sg{t}")
                nc.vector.tensor_mul(Sg[:], gn_sb[:], rQ[:])

                X = wp.tile([P, BPC], f32, tag="X", name=f"X{t}", bufs=2)
                nc.vector.tensor_mul(X[:], t1_ps[:], fz_sb[:])
                rX = wp.tile([P, BPC], f32, tag="rX", name=f"rX{t}", bufs=1)
                _act_raw(nc, mybir, AF.Rsqrt, rX[:], X[:])
                st = wp.tile([P, BPC], f32, tag="st", name=f"st{t}", bufs=2)
                nc.vector.tensor_mul(st[:], t1_ps[:], rX[:])
                prev_st = st
                usq = wp.tile([P, BPC], f32, tag="usq", name=f"usq{t}", bufs=1)
                _act_raw(nc, mybir, AF.Square, usq[:], st[:], scale=1.0, bias=1.0)
                rden = wp.tile([P, BPC], f32, tag="rden", name=f"rden{t}", bufs=1)
                _act_raw(nc, mybir, AF.Rsqrt, rden[:], usq[:])
                # E = mu/(1+sqrt(t1/fz)) with mu = W4*fs exact from the PE
                # (an fz - t1 subtraction would be pure f32r noise at y->1)
                E = wp.tile([P, BPC], bf16, tag="E", name=f"E{t}", bufs=2)
                nc.vector.tensor_mul(E[:], mu_sb[:], rden[:])
                D = wp.tile([P, BPC], bf16, tag="D", name=f"D{t}", bufs=2)
                nc.vector.tensor_mul(D[:], Sg[:], E[:])

                wt_r = iop.tile([P, 3], bf16, tag="wt_r")
                nc.vector.tensor_copy(out=wt_r[:], in_=rw[:, 3 * t:3 * t + 3])

                for h in range(2):
                    cs = slice(h * HALF, (h + 1) * HALF)
                    nc.tensor.matmul(acc[0:1, cs], wt_r[:, 0:1], Sg[:, cs],
                                     start=first, stop=last, skip_group_check=True)
                    nc.tensor.matmul(acc[32:33, cs], wt_r[:, 1:2], D[:, cs],
                                     start=first, stop=last, skip_group_check=True)

                # ---- disconnected grid (y2): zd, fzd, gnd via PE ----
                zd_ps = pp.tile([P, BPC], f32, tag="poly", name=f"zd_ps{t}")
                fzd_ps = pp.tile([P, BPC], f32, tag="poly", name=f"fzd_ps{t}")
                gnd_ps = pp.tile([P, BPC], f32, tag="poly", name=f"gnd_ps{t}")
                for h in range(2):
                    cs = slice(h * HALF, (h + 1) * HALF)
                    rv = zrd[:, cs]
                    nc.tensor.matmul(zd_ps[:, cs], blk(4, t), rv, start=True, stop=True)
                    nc.tensor.matmul(fzd_ps[:, cs], blk(5, t), rv, start=True, stop=True)
                    nc.tensor.matmul(gnd_ps[:, cs], blk(6, t), rv, start=True, stop=True)

                zd2 = wp.tile([P, BPC], f32, tag="zd2", name=f"zd2{t}", bufs=1)
                nc.scalar.square(out=zd2[:], in_=zd_ps[:])
                zd4 = wp.tile([P, BPC], f32, tag="zd4", name=f"zd4{t}", bufs=1)
                nc.scalar.square(out=zd4[:], in_=zd2[:])
                gnd_sb = wp.tile([P, BPC], f32, tag="gnd_sb", name=f"gnd_sb{t}")
                nc.scalar.copy(out=gnd_sb[:], in_=gnd_ps[:])

                gdd = wp.tile([P, BPC], f32, tag="gdd", name=f"gdd{t}", bufs=1)
                nc.vector.tensor_scalar(out=gdd[:], in0=zd4[:], scalar1=-1.0,
                                        scalar2=1.0, op0=mybir.AluOpType.mult,
                                        op1=mybir.AluOpType.add)
                Bt = wp.tile([P, BPC], f32, tag="Bt", name=f"Bt{t}", bufs=1)
                bt_abs = nc.gpsimd.tensor_copy(out=tinyp[:], in_=gdd[0:1, 0:1])
                bt_i = nc.gpsimd.tensor_mul(Bt[:], gdd[:], zd4[:])
                _add_dep_helper(bt_i.ins, bt_abs.ins, sync=False,
                                reason="pool absorber order")
                Pt = wp.tile([P, BPC], f32, tag="Pt", name=f"Pt{t}")
                nc.vector.tensor_mul(Pt[:], fzd_ps[:], gnd_sb[:])
                PBt = wp.tile([P, BPC], f32, tag="PBt", name=f"PBt{t}", bufs=1)
                pb_abs = nc.gpsimd.tensor_copy(out=tinyp[:], in_=Pt[0:1, 0:1])
                pb_i = nc.gpsimd.tensor_mul(PBt[:], Pt[:], Bt[:])
                _add_dep_helper(pb_i.ins, pb_abs.ins, sync=False,
                                reason="pool absorber order")
                r2 = wp.tile([P, BPC], f32, tag="r2", name=f"r2{t}", bufs=1)
                _act_raw(nc, mybir, AF.Rsqrt, r2[:], PBt[:])
                S3 = wp.tile([P, BPC], bf16, tag="S3", name=f"S3{t}", bufs=1)
                nc.vector.tensor_mul(S3[:], Pt[:], r2[:])

                for h in range(2):
                    cs = slice(h * HALF, (h + 1) * HALF)
                    nc.tensor.matmul(acc[64:65, cs], wt_r[:, 2:3], S3[:, cs],
                                     start=first, stop=last, skip_group_check=True)

            # ---- tail: gather accumulator rows to partition base 0 with
            # selector matmuls, scale, and write out ----
            sums_w = cp.tile([65, BPC], f32)
            nc.scalar.copy(out=sums_w[:], in_=acc[:])
            s2_ps = pp.tile([2, BPC], f32, tag="poly", name="s2_ps")
            sc_ps = pp.tile([2, BPC], f32, tag="poly", name="sc_ps")
            for h in range(2):
                cs = slice(h * HALF, (h + 1) * HALF)
                nc.tensor.matmul(s2_ps[:, cs], rw[0:65, NT * 3:NT * 3 + 2],
                                 sums_w[:, cs], start=True, stop=True)
                nc.tensor.matmul(sc_ps[:, cs], rw[0:65, NT * 3 + 2:NT * 3 + 4],
                                 sums_w[:, cs], start=True, stop=True)
            sums2 = cp.tile([2, BPC], f32)
            nc.scalar.copy(out=sums2[:], in_=s2_ps[:])
            scr = cp.tile([2, BPC], f32)
            nc.scalar.copy(out=scr[:], in_=sc_ps[:])
            pr = cp.tile([2, BPC], f32)
            nc.vector.tensor_mul(pr[:], sums2[:], sr[:, 0:BPC])
            nc.vector.tensor_mul(scr[:], scr[:], sr[:, BPC:2 * BPC])
            nc.vector.tensor_add(pr[:], pr[:], scr[:])
            nc.vector.tensor_add(pr[:], pr[:], sr[:, 2 * BPC:3 * BPC])
            nc.sync.dma_start(out=out_d[:], in_=pr[:])

    _prune_redundant_waits(nc)
    return nc


def _get_nc():
    if "nc" not in _COMPILED:
        _COMPILED["nc"] = _build_nc()
    return _COMPILED["nc"]


def kernel(a, b, logcoef, shift, zs, _trace=False):
    from concourse.bass_utils import run_bass_kernel_spmd

    a = np.asarray(a)
    b = np.asarray(b)
    zs = np.asarray(zs)
    assert zs.shape == (B_TOTAL,)

    lcoef_t, rwts, zrows_all, srows_all = _build_host_tables(a, b, logcoef, shift, zs)

    in_maps = [
        {
            "zrows": zrows_all[c],
            "lcoef": lcoef_t,
            "rwts": rwts,
            "srows": srows_all[c],
        }
        for c in range(NCORES)
    ]

    nc = _get_nc()
    res = run_bass_kernel_spmd(nc, in_maps, core_ids=list(range(NCORES)),
                               trace=_trace)
    out = np.concatenate([res.results[c]["out"] for c in range(NCORES)], axis=1)
    if _trace:
        kernel.last_exec_time_ns = res.exec_time_ns
        kernel.last_profile = res.profile_json
    return out.astype(np.float32)
